# revision 4
# baseline (speedup 1.0000x reference)
"""GQA inference attention block on 8 Trainium2 NeuronCores.

Sharding: core c = (batch b = c//2, head-half s = c%2).
Each core handles 1 batch, 16 Q heads, 4 KV heads (GQA groups intact:
Q head h uses KV head h % 8; half s owns heads with h % 8 in [4s, 4s+4)).

Per-core device program (bf16 matmuls, f32 accumulation):
  1. QT = Wq_c @ hidden[b]^T    [2048, 1024]   (d-major layout for attention)
     KT = Wk_c @ hidden[b]^T    [512, 1024]
     V  = hidden[b] @ Wv_c^T    [1024, 512]    (k-major layout for PV matmul)
  2. Attention per (head, 512-wide q block): S^T tiles [128 k, 512 q] on PE,
     additive causal masks (tril diagonal=1 quirk) + padding bias folded into
     ACT exp, P^T bf16 feeds PV matmul (out^T accum) and a ones-matmul
     (denominator broadcast across partitions). No transposes anywhere.
  3. out_partial = attn_out @ Wo_c^T  [1024, 4096] f32, DMA'd out.
Host: out[b] = out_partial[core 2b] + out_partial[core 2b+1].
"""

import sys

if "/opt/trn_rl_repo" not in sys.path:
    sys.path.insert(0, "/opt/trn_rl_repo")

import numpy as np

import concourse.bass as bass
import concourse.tile as tile
from concourse import bacc, mybir
from concourse.bass_utils import run_bass_kernel_spmd

F32 = mybir.dt.float32
BF16 = mybir.dt.bfloat16
NP_BF16 = mybir.dt.np(BF16)

HID = 4096
T = 1024
B = 4
NH = 32
NKV = 8
HD = 128
NEG = -1e30
SCALE = 1.0 / np.sqrt(HD)

# per-core
LH = 16     # local Q heads
LKV = 4     # local KV heads
DQ = LH * HD      # 2048
DKV = LKV * HD    # 512
KH = HID // 128   # 32 hidden k-tiles
TT = T // 128     # 8 token tiles
TB = T // 512     # 2 token blocks
EB = HID // 512   # 8 output column blocks
KM = DQ // 128    # 16 contraction tiles for Wo

_CACHE = {}


def _build_masks():
    """5 additive mask patterns [128 k, 512 q] for the diagonal k-tiles.

    For q block I (q = 512*I + 128*r + qr) and k tile j = 4*I + s:
      sub-tile r vs s:  r > s: open;  r == s: T0 (k <= q+1 within tile);
      r == s-1: T1 (only kc==0, qr==127 open);  r < s-1: fully masked.
    s = 4 (j = 4I+4) only exists for I=0: [masked, masked, masked, T1].
    """
    kc = np.arange(128)[:, None]
    qr = np.arange(128)[None, :]
    t0 = np.where(kc <= qr + 1, 0.0, NEG).astype(np.float32)
    t1 = np.full((128, 128), NEG, np.float32)
    t1[0, 127] = 0.0
    zero = np.zeros((128, 128), np.float32)
    full = np.full((128, 128), NEG, np.float32)
    masks = np.zeros((5, 128, 512), np.float32)
    for s in range(5):
        blocks = []
        for r in range(4):
            if r > s:
                blocks.append(zero)
            elif r == s:
                blocks.append(t0)
            elif r == s - 1:
                blocks.append(t1)
            else:
                blocks.append(full)
        masks[s] = np.concatenate(blocks, axis=1)
    return masks


def _build_program():
    nc = bacc.Bacc("TRN2", target_bir_lowering=False, debug=False)

    hT_d = nc.dram_tensor("hT", [KH, 128, T], BF16, kind="ExternalInput")
    wq_d = nc.dram_tensor("wq", [KM, 128, KH, 128], BF16, kind="ExternalInput")
    wk_d = nc.dram_tensor("wk", [LKV, 128, KH, 128], BF16, kind="ExternalInput")
    wv_d = nc.dram_tensor("wv", [KH, 128, DKV], BF16, kind="ExternalInput")
    wo_d = nc.dram_tensor("wo", [EB, 128, KM, 512], BF16, kind="ExternalInput")
    masks_d = nc.dram_tensor("masks", [5, 128, 512], F32, kind="ExternalInput")
    kpad_d = nc.dram_tensor("kpad", [128, TT], F32, kind="ExternalInput")
    out_d = nc.dram_tensor("out", [T, HID], F32, kind="ExternalOutput")

    with tile.TileContext(nc) as tc:
        with (
            tc.tile_pool(name="persist", bufs=1) as persist,
        ):
            qt_sb = persist.tile([128, KM * T], BF16, tag="qt")
            kt_sb = persist.tile([128, LKV * T], BF16, tag="kt")
            v_sb = persist.tile([128, LKV * TT * 128], BF16, tag="v")
            masks_sb = persist.tile([128, 5 * 512], F32, tag="masks")
            kpad_sb = persist.tile([128, TT], F32, tag="kpad")
            ones_sb = persist.tile([128, 128], BF16, tag="ones")

            nc.sync.dma_start(
                masks_sb[:].rearrange("p (s n) -> p s n", s=5),
                masks_d.ap().rearrange("s p n -> p s n"),
            )
            nc.sync.dma_start(kpad_sb[:], kpad_d.ap())
            nc.vector.memset(ones_sb[:], 1.0)

            # ---- phase 1: projections ----
            with (
                tc.tile_pool(name="ht", bufs=1) as ht_pool,
                tc.tile_pool(name="w_lhs", bufs=2) as w_lhs,
                tc.tile_pool(name="wv_rhs", bufs=1) as wv_rhs,
                tc.tile_pool(name="psum1", bufs=4, space="PSUM") as psum1,
            ):
                ht_sb = ht_pool.tile([128, KH * T], BF16, tag="ht")
                for kh in range(KH):
                    nc.sync.dma_start(
                        ht_sb[:, kh * T:(kh + 1) * T], hT_d.ap()[kh]
                    )

                # V = hidden @ Wv^T  -> [t, dv] layout (k-tokens on partitions)
                wv_sb = wv_rhs.tile([128, KH * DKV], BF16, tag="wv")
                nc.sync.dma_start(
                    wv_sb[:].rearrange("p (kh n) -> p kh n", kh=KH),
                    wv_d.ap().rearrange("kh p n -> p kh n"),
                )
                for tt in range(TT):
                    ps = psum1.tile([128, DKV], F32, tag="ps1")
                    for kh in range(KH):
                        nc.tensor.matmul(
                            ps[:],
                            ht_sb[:, kh * T + tt * 128: kh * T + (tt + 1) * 128],
                            wv_sb[:, kh * DKV:(kh + 1) * DKV],
                            start=(kh == 0),
                            stop=(kh == KH - 1),
                        )
                    for g in range(LKV):
                        nc.scalar.copy(
                            v_sb[:, (g * TT + tt) * 128:(g * TT + tt + 1) * 128],
                            ps[:, g * 128:(g + 1) * 128],
                        )

                # KT = Wk @ hidden^T -> [dk, t]
                for dkt in range(LKV):
                    wkt = w_lhs.tile([128, KH * 128], BF16, tag="wlhs")
                    nc.sync.dma_start(
                        wkt[:].rearrange("p (kh d) -> p kh d", kh=KH),
                        wk_d.ap()[dkt],
                    )
                    for tb in range(TB):
                        ps = psum1.tile([128, 512], F32, tag="ps1")
                        for kh in range(KH):
                            nc.tensor.matmul(
                                ps[:],
                                wkt[:, kh * 128:(kh + 1) * 128],
                                ht_sb[:, kh * T + tb * 512: kh * T + tb * 512 + 512],
                                start=(kh == 0),
                                stop=(kh == KH - 1),
                            )
                        nc.scalar.copy(
                            kt_sb[:, dkt * T + tb * 512: dkt * T + tb * 512 + 512],
                            ps[:],
                        )

                # QT = Wq @ hidden^T -> [dq, t]
                for dqt in range(KM):
                    wqt = w_lhs.tile([128, KH * 128], BF16, tag="wlhs")
                    nc.sync.dma_start(
                        wqt[:].rearrange("p (kh d) -> p kh d", kh=KH),
                        wq_d.ap()[dqt],
                    )
                    for tb in range(TB):
                        ps = psum1.tile([128, 512], F32, tag="ps1")
                        for kh in range(KH):
                            nc.tensor.matmul(
                                ps[:],
                                wqt[:, kh * 128:(kh + 1) * 128],
                                ht_sb[:, kh * T + tb * 512: kh * T + tb * 512 + 512],
                                start=(kh == 0),
                                stop=(kh == KH - 1),
                            )
                        nc.scalar.copy(
                            qt_sb[:, dqt * T + tb * 512: dqt * T + tb * 512 + 512],
                            ps[:],
                        )

            # ---- phases 2+3 share aot ----
            with tc.tile_pool(name="aot_pool", bufs=1) as aot_pool:
              aot_sb = aot_pool.tile([128, KM * T], BF16, tag="aot")
              # ---- phase 2: attention ----
              with (
                tc.tile_pool(name="pt", bufs=6) as pt_pool,
                tc.tile_pool(name="rec", bufs=2) as rec_pool,
                tc.tile_pool(name="psum_st", bufs=4, space="PSUM") as psum_st,
                tc.tile_pool(name="psum_od", bufs=2, space="PSUM") as psum_od,
              ):
                for jh in range(LH):
                    g = jh // 4
                    for I in range(TB):
                        njt = 4 * I + 5 if I < TB - 1 else TT
                        ot = psum_od.tile([128, 512], F32, tag="ot")
                        den = psum_od.tile([128, 512], F32, tag="den")
                        for j in range(njt):
                            st = psum_st.tile([128, 512], F32, tag="st")
                            nc.tensor.matmul(
                                st[:],
                                kt_sb[:, (g * TT + j) * 128:(g * TT + j + 1) * 128],
                                qt_sb[:, jh * T + I * 512: jh * T + I * 512 + 512],
                                start=True,
                                stop=True,
                            )
                            s = j - 4 * I
                            if s >= 0:
                                nc.vector.tensor_add(
                                    st[:], st[:], masks_sb[:, s * 512:(s + 1) * 512]
                                )
                            pt = pt_pool.tile([128, 512], BF16, tag="pt")
                            nc.scalar.activation(
                                pt[:],
                                st[:],
                                mybir.ActivationFunctionType.Exp,
                                bias=kpad_sb[:, j:j + 1],
                                scale=SCALE,
                            )
                            nc.tensor.matmul(
                                ot[:],
                                v_sb[:, (g * TT + j) * 128:(g * TT + j + 1) * 128],
                                pt[:],
                                start=(j == 0),
                                stop=(j == njt - 1),
                            )
                            nc.tensor.matmul(
                                den[:],
                                ones_sb[:],
                                pt[:],
                                start=(j == 0),
                                stop=(j == njt - 1),
                            )
                        rec = rec_pool.tile([128, 512], F32, tag="rec")
                        nc.vector.reciprocal(rec[:], den[:])
                        nc.vector.tensor_mul(
                            aot_sb[:, jh * T + I * 512: jh * T + I * 512 + 512],
                            ot[:],
                            rec[:],
                        )

              # ---- phase 3: output projection partial ----
              with (
                tc.tile_pool(name="wo_rhs", bufs=2) as wo_rhs,
                tc.tile_pool(name="ob", bufs=4) as ob_pool,
                tc.tile_pool(name="psum3", bufs=4, space="PSUM") as psum3,
            ):
                for eb in range(EB):
                    wot = wo_rhs.tile([128, KM * 512], BF16, tag="wo")
                    nc.sync.dma_start(
                        wot[:].rearrange("p (km n) -> p km n", km=KM),
                        wo_d.ap()[eb],
                    )
                    for tt in range(TT):
                        ps = psum3.tile([128, 512], F32, tag="ps3")
                        for km in range(KM):
                            nc.tensor.matmul(
                                ps[:],
                                aot_sb[:, km * T + tt * 128: km * T + (tt + 1) * 128],
                                wot[:, km * 512:(km + 1) * 512],
                                start=(km == 0),
                                stop=(km == KM - 1),
                            )
                        ob = ob_pool.tile([128, 512], F32, tag="ob")
                        nc.scalar.copy(ob[:], ps[:])
                        nc.sync.dma_start(
                            out_d.ap()[tt * 128:(tt + 1) * 128, eb * 512:(eb + 1) * 512],
                            ob[:],
                        )

    nc.compile()
    return nc


def _get_program():
    if "nc" not in _CACHE:
        _CACHE["nc"] = _build_program()
    return _CACHE["nc"]


def _core_heads(s):
    return [8 * r + 4 * s + g for g in range(4) for r in range(4)]


def _prep_core_inputs(input_ids, hidden_state, Wq, Wk, Wv, Wo, masks):
    """Build the 8 per-core input maps (host-side slicing/transpose/cast)."""
    in_maps = []
    for c in range(8):
        b, s = c // 2, c % 2
        heads = _core_heads(s)
        kv_heads = [4 * s + g for g in range(4)]

        hT = np.ascontiguousarray(
            hidden_state[b].reshape(T, KH, 128).transpose(1, 2, 0)
        ).astype(NP_BF16)

        wq_c = Wq.reshape(NH, HD, HID)[heads]          # [16, 128, 4096]
        wq = np.ascontiguousarray(
            wq_c.reshape(KM, HD, KH, 128).transpose(0, 3, 2, 1)
        ).astype(NP_BF16)                              # [16, 128p, 32kh, 128d]

        wk_c = Wk.reshape(NKV, HD, HID)[kv_heads]      # [4, 128, 4096]
        wk = np.ascontiguousarray(
            wk_c.reshape(LKV, HD, KH, 128).transpose(0, 3, 2, 1)
        ).astype(NP_BF16)

        wv_c = Wv.reshape(NKV, HD, HID)[kv_heads]      # [4, 128, 4096]
        wv = np.ascontiguousarray(
            wv_c.reshape(LKV, HD, KH, 128).transpose(2, 3, 0, 1).reshape(KH, 128, DKV)
        ).astype(NP_BF16)                              # [32kh, 128p, 512dv]

        wo_c = Wo.reshape(HID, NH, HD)[:, heads, :].reshape(HID, DQ)
        wo = np.ascontiguousarray(
            wo_c.reshape(EB, 512, KM, 128).transpose(0, 3, 2, 1)
        ).astype(NP_BF16)                              # [8eb, 128p, 16km, 512e]

        kpad = np.where(input_ids[b] != 0, 0.0, NEG).astype(np.float32)
        kpad = np.ascontiguousarray(kpad.reshape(TT, 128).T)  # [128, 8]

        in_maps.append(
            {
                "hT": hT,
                "wq": wq,
                "wk": wk,
                "wv": wv,
                "wo": wo,
                "masks": masks,
                "kpad": kpad,
            }
        )
    return in_maps


def kernel(input_ids, hidden_state, Wq, Wk, Wv, Wo, k_cache, v_cache, start_pos):
    input_ids = np.asarray(input_ids)
    hidden_state = np.asarray(hidden_state, dtype=np.float32)
    Wq = np.asarray(Wq, dtype=np.float32)
    Wk = np.asarray(Wk, dtype=np.float32)
    Wv = np.asarray(Wv, dtype=np.float32)
    Wo = np.asarray(Wo, dtype=np.float32)

    nc = _get_program()
    in_maps = _prep_core_inputs(
        input_ids, hidden_state, Wq, Wk, Wv, Wo, _build_masks()
    )
    res = run_bass_kernel_spmd(nc, in_maps, list(range(8)))
    out = np.empty((B, T, HID), np.float32)
    for b in range(B):
        out[b] = res.results[2 * b]["out"] + res.results[2 * b + 1]["out"]
    return out, int(start_pos) + input_ids.shape[1]
